# revision 1
# baseline (speedup 1.0000x reference)
"""Trainium2 Bass kernel for nn_BoxRepelLoss (rotated-box repel/IoU loss).

Math: replaces the reference's convex-hull-by-argsort intersection area with
an equivalent sort-free Green's-theorem form. For convex CCW polygons P, Q:

    2*Area(P inter Q) = sum over the 8 edges (4 of P Liang-Barsky-clipped
    against Q's slab half-planes, 4 of Q against P's) of
    (t_hi - t_lo) * cross(a, b - a),  t clamped to [0, 1]

since each clipped segment's line-integral contribution collapses to
dt * cross(a, e). All per-pair work is elementwise -> Vector engine.

Pair enumeration (halves work vs the full [m, m] grid): unordered pairs
(i, (i+k) mod m), k = 1..m/2; the k = m/2 row appears on two cores and is
weighted 0.5 on both (bitwise-identical values, so the sum stays exact).
Grid layout per core: partition p x free (kt, c), with k = kt*128 + p + 1
(kt = 0..2) and i = 96*d + c (c = 0..95) -- core d owns a 96-column i-slab.
Features reach each core as:
  - "peri" [NR, 288]      per-i rows (pre-replicated x3), partition-broadcast
  - "hank" [NR*3, 224]    sliding windows of the wrap-extended feature arrays;
                          partner j = i + k features materialize via Hankel
                          access patterns f[p + 1 + kt*128 + c]
Both directions' edge quantities live in one 8-slot [128, 8*288] layout
(slots = 4 edges x 2 directions) so the Liang-Barsky interval math runs as
~2300-wide DVE ops. Edge projections come from corner-projection differences
(r[e] = dca[(e+1)%4] - dca[e]); interval endpoints use
lo = -w2*|1/r| - dca/r, hi = +w2*|1/r| - dca/r (no root sort needed).

Each core emits partial sums (S_iou, S_rep, S_size); the host combines:
  total = 2*S_rep/(m(m-1)) + S_size/m + 2*S_iou/m^2
"""

import numpy as np

M = 768
NDEV = 8
CPD = M // NDEV          # 96 i-columns per core
NKT = 3                  # k-tiles: k = kt*128 + p + 1 in [1, 384]
W288 = NKT * CPD         # 288 pair-columns per partition
W1152 = 4 * W288         # one direction, 4 edge slots
W2304 = 2 * W1152        # both directions
HROW = 224               # hankel window row length (per (kt,r) row)

# feature-row indices (same semantics in peri and hank)
R_XA, R_YA, R_K = 0, 4, 8
R_COS, R_SIN, R_UC, R_US, R_W2, R_H2 = 12, 13, 14, 15, 16, 17
R_CX, R_CY, R_A2, R_WCOL = 18, 19, 20, 21
NR = 22

REPEL_MARGIN = 0.08
MIN_SIZE = 0.02
IOU_MARGIN = 0.1

_PROGRAM_CACHE = {}


def _features(pred):
    """Per-box feature table F [NR-1, M] (fp32, matching reference math)."""
    p = np.asarray(pred, np.float32)[:-1]
    cx, cy, w, h = p[:, 0], p[:, 1], p[:, 2], p[:, 3]
    th = np.arctan2(p[:, 5], p[:, 4]).astype(np.float32)
    c = np.cos(th).astype(np.float32)
    s = np.sin(th).astype(np.float32)
    dx = np.stack([-w, w, w, -w], 0) * np.float32(0.5)   # [4, M]
    dy = np.stack([-h, -h, h, h], 0) * np.float32(0.5)
    xa = cx[None] + c[None] * dx - s[None] * dy           # [4, M]
    ya = cy[None] + s[None] * dx + c[None] * dy
    ex = np.roll(xa, -1, 0) - xa
    ey = np.roll(ya, -1, 0) - ya
    K = xa * ey - ya * ex
    F = np.empty((NR - 1, M), np.float32)
    F[R_XA:R_XA + 4] = xa
    F[R_YA:R_YA + 4] = ya
    F[R_K:R_K + 4] = K
    F[R_COS], F[R_SIN] = c, s
    F[R_UC] = c * cx + s * cy
    F[R_US] = -s * cx + c * cy
    F[R_W2], F[R_H2] = w * 0.5, h * 0.5
    F[R_CX], F[R_CY] = cx, cy
    F[R_A2] = 2.0 * w * h
    return F


# DMA row groups in consumption order: the first A-phase ops need only
# cos/sin/uc/us (clip) + xa/ya (subject); w2..wcol feed B and the epilogue;
# K rows are only needed by the C phase.
_GROUPS = [(R_COS, R_W2), (R_XA, R_K), (R_W2, NR), (R_K, R_COS)]


def _build_program():
    import concourse.bass as bass
    import concourse.mybir as mybir
    from concourse import bacc
    from concourse.tile import TileContext

    fp32 = mybir.dt.float32
    Alu = mybir.AluOpType
    Act = mybir.ActivationFunctionType

    nc = bacc.Bacc('TRN2', target_bir_lowering=False, debug=False)
    for v in (REPEL_MARGIN, MIN_SIZE):
        t = nc.alloc_sbuf_tensor(f'const-f32-{v}', [128, 1], fp32)
        nc.gpsimd.memset(t.ap(), v)
        nc.const_aps.aps[(fp32, v)] = t.ap()
    nc.all_engine_barrier()

    hank = nc.dram_tensor('hank', [NR * NKT, HROW], fp32, kind='ExternalInput')
    peri = nc.dram_tensor('peri', [NR, W288], fp32, kind='ExternalInput')
    out = nc.dram_tensor('out', [4, 1], fp32, kind='ExternalOutput')

    def sub(t, off, free_dims):
        base = t[:]
        return bass.AP(base.tensor, base.offset + off, [list(base.ap[0])] + free_dims)

    with TileContext(nc) as tc:
        with tc.tile_pool(name='p', bufs=1) as pool, \
             tc.tile_pool(name='ps', bufs=1, space='PSUM') as ppool:
            psum4 = ppool.tile([4, 1], fp32, tag='psum4')
            hank_sb = pool.tile([128, NR * W288], fp32, tag='hank')
            peri_sb = pool.tile([128, NR * W288], fp32, tag='peri')

            hout, pout = hank_sb[:], peri_sb[:]
            for (a, b) in _GROUPS:
                n = b - a
                nc.sync.dma_start(
                    out=bass.AP(hout.tensor, hout.offset + a * W288,
                                [list(hout.ap[0]), [CPD, n * NKT], [1, CPD]]),
                    in_=bass.AP(hank[:].tensor, a * NKT * HROW + 1,
                                [[1, 128], [HROW, n * NKT], [1, CPD]]))
                nc.sync.dma_start(
                    out=bass.AP(pout.tensor, pout.offset + a * W288,
                                [list(pout.ap[0]), [1, n * W288]]),
                    in_=bass.AP(peri[:].tensor, a * W288,
                                [[0, 128], [1, n * W288]]))

            def crow(bank, r):   # clip row, e-broadcast [128, 4, 288]
                return sub(bank, r * W288, [[0, 4], [1, W288]])

            def v4(bank, r0):    # 4-row block as [128, 4, 288]
                return sub(bank, r0 * W288, [[W288, 4], [1, W288]])

            def flat4(bank, r0):  # 4-row block as [128, 1152]
                return sub(bank, r0 * W288, [[1, W1152]])

            def frow(bank, r):   # single row [128, 288]
                return sub(bank, r * W288, [[1, W288]])

            wcol = sub(hank_sb, R_WCOL * W288, [[1, 1]])

            def wt(tag):
                return pool.tile([128, W2304], fp32, tag=tag, name=tag)

            dca_c, dca_s = wt('dca_c'), wt('dca_s')
            r_c, r_s = wt('r_c'), wt('r_s')
            scr, t1, t2 = wt('scr'), wt('t1'), wt('t2')
            S = pool.tile([128, W288], fp32, tag='S')
            U = pool.tile([128, W288], fp32, tag='U')
            R = pool.tile([128, W288], fp32, tag='R')
            X1 = pool.tile([128, W288], fp32, tag='X1')
            X2 = pool.tile([128, W288], fp32, tag='X2')
            z96a = pool.tile([1, CPD], fp32, tag='z96a')
            z96b = pool.tile([1, CPD], fp32, tag='z96b')
            acc4 = pool.tile([128, 4], fp32, tag='acc4')
            red4 = pool.tile([128, 4], fp32, tag='red4')
            ones = pool.tile([128, 1], fp32, tag='ones')

            tt = nc.vector.tensor_tensor
            ts = nc.vector.tensor_scalar
            stt = nc.vector.scalar_tensor_tensor

            def half4(t, ho):    # one direction half viewed [128, 4, 288]
                return sub(t, ho, [[W288, 4], [1, W288]])

            def seg(t, lo, hi):  # flat column range
                return t[:, lo:hi]

            # ---- A phase: corner projections + edge projections ----
            # (measured: GpSimd TT is ~3x slower than DVE here, so offloading
            # one direction to it lengthens the critical path -- keep all DVE)
            for ho, subj, clip in ((0, peri_sb, hank_sb), (W1152, hank_sb, peri_sb)):
                Cc, Cs = crow(clip, R_COS), crow(clip, R_SIN)
                tt(out=half4(scr, ho), in0=Cc, in1=v4(subj, R_XA), op=Alu.mult)
                tt(out=half4(t1, ho), in0=Cs, in1=v4(subj, R_YA), op=Alu.mult)
                tt(out=seg(scr, ho, ho + W1152), in0=seg(scr, ho, ho + W1152),
                   in1=seg(t1, ho, ho + W1152), op=Alu.add)
                tt(out=half4(dca_c, ho), in0=half4(scr, ho),
                   in1=crow(clip, R_UC), op=Alu.subtract)
                tt(out=half4(scr, ho), in0=Cc, in1=v4(subj, R_YA), op=Alu.mult)
                tt(out=half4(t1, ho), in0=Cs, in1=v4(subj, R_XA), op=Alu.mult)
                tt(out=seg(scr, ho, ho + W1152), in0=seg(scr, ho, ho + W1152),
                   in1=seg(t1, ho, ho + W1152), op=Alu.subtract)
                tt(out=half4(dca_s, ho), in0=half4(scr, ho),
                   in1=crow(clip, R_US), op=Alu.subtract)
                # edge projections r[e] = dca[(e+1)%4] - dca[e]
                for dca, rr in ((dca_c, r_c), (dca_s, r_s)):
                    tt(out=seg(rr, ho, ho + 3 * W288),
                       in0=seg(dca, ho + W288, ho + W1152),
                       in1=seg(dca, ho, ho + 3 * W288), op=Alu.subtract)
                    tt(out=seg(rr, ho + 3 * W288, ho + W1152),
                       in0=seg(dca, ho, ho + W288),
                       in1=seg(dca, ho + 3 * W288, ho + W1152), op=Alu.subtract)

            # ---- B phase (both directions fused, 2304-wide) ----
            # h = w2 * rinv; habs = max(h, -h); hi = habs - g; lo = -habs - g
            for dca, rr, w2r, habs, lo_dst in (
                    (dca_c, r_c, R_W2, t2, t2),
                    (dca_s, r_s, R_H2, dca_c, t1)):
                nc.vector.reciprocal_approx_fast(out=t1[:], in_=rr[:])
                tt(out=scr[:], in0=dca[:], in1=t1[:], op=Alu.mult)
                tt(out=half4(rr, 0), in0=crow(hank_sb, w2r),
                   in1=half4(t1, 0), op=Alu.mult)
                tt(out=half4(rr, W1152), in0=crow(peri_sb, w2r),
                   in1=half4(t1, W1152), op=Alu.mult)
                stt(out=habs[:], in0=rr[:], scalar=-1.0, in1=rr[:],
                    op0=Alu.mult, op1=Alu.max)
                tt(out=rr[:], in0=habs[:], in1=scr[:], op=Alu.subtract)
                stt(out=lo_dst[:], in0=habs[:], scalar=-1.0, in1=scr[:],
                    op0=Alu.mult, op1=Alu.subtract)

            # ---- C phase: clamp, dt, weight by cross const, reduce ----
            tt(out=t1[:], in0=t2[:], in1=t1[:], op=Alu.max)        # LO
            ts(out=t1[:], in0=t1[:], scalar1=0.0, scalar2=1.0,
               op0=Alu.max, op1=Alu.min)
            tt(out=r_c[:], in0=r_c[:], in1=r_s[:], op=Alu.min)     # HI
            ts(out=r_c[:], in0=r_c[:], scalar1=0.0, scalar2=1.0,
               op0=Alu.max, op1=Alu.min)
            tt(out=t1[:], in0=r_c[:], in1=t1[:], op=Alu.subtract)  # dt
            ts(out=t1[:], in0=t1[:], scalar1=0.0, scalar2=None, op0=Alu.max)
            tt(out=seg(t1, 0, W1152), in0=seg(t1, 0, W1152),
               in1=flat4(peri_sb, R_K), op=Alu.mult)
            tt(out=seg(t1, W1152, W2304), in0=seg(t1, W1152, W2304),
               in1=flat4(hank_sb, R_K), op=Alu.mult)
            tt(out=seg(t1, 0, W1152), in0=seg(t1, 0, W1152),
               in1=seg(t1, W1152, W2304), op=Alu.add)
            tt(out=seg(t1, 0, 2 * W288), in0=seg(t1, 0, 2 * W288),
               in1=seg(t1, 2 * W288, W1152), op=Alu.add)
            tt(out=S[:], in0=seg(t1, 0, W288), in1=seg(t1, W288, 2 * W288),
               op=Alu.add)

            # ---- IoU epilogue ----
            tt(out=U[:], in0=frow(peri_sb, R_A2), in1=frow(hank_sb, R_A2),
               op=Alu.add)
            tt(out=U[:], in0=U[:], in1=S[:], op=Alu.subtract)      # union2
            nc.vector.reciprocal_approx_fast(out=R[:], in_=U[:])
            tt(out=R[:], in0=S[:], in1=R[:], op=Alu.mult)          # iou
            ts(out=R[:], in0=R[:], scalar1=IOU_MARGIN, scalar2=0.0,
               op0=Alu.subtract, op1=Alu.max)
            nc.vector.memset(acc4[:], 0.0)
            ts(out=R[:, 2 * CPD:W288], in0=R[:, 2 * CPD:W288],
               scalar1=wcol, scalar2=None, op0=Alu.mult)
            nc.vector.tensor_reduce(out=acc4[:, 0:1], in_=R[:],
                                    axis=mybir.AxisListType.X, op=Alu.add)

            # ---- repel ----
            tt(out=X1[:], in0=frow(hank_sb, R_CX), in1=frow(peri_sb, R_CX),
               op=Alu.subtract)
            tt(out=X2[:], in0=frow(hank_sb, R_CY), in1=frow(peri_sb, R_CY),
               op=Alu.subtract)
            tt(out=X1[:], in0=X1[:], in1=X1[:], op=Alu.mult)
            tt(out=X2[:], in0=X2[:], in1=X2[:], op=Alu.mult)
            tt(out=X1[:], in0=X1[:], in1=X2[:], op=Alu.add)
            nc.scalar.activation(out=X1[:], in_=X1[:], func=Act.Sqrt)
            nc.scalar.activation(out=X1[:], in_=X1[:], func=Act.Relu,
                                 bias=REPEL_MARGIN, scale=-1.0)
            ts(out=X1[:, 2 * CPD:W288], in0=X1[:, 2 * CPD:W288],
               scalar1=wcol, scalar2=None, op0=Alu.mult)
            nc.vector.tensor_reduce(out=acc4[:, 1:2], in_=X1[:],
                                    axis=mybir.AxisListType.X, op=Alu.add)

            # ---- size penalty (this core's 96 boxes) ----
            nc.scalar.activation(out=z96a[:],
                                 in_=peri_sb[0:1, R_W2 * W288:R_W2 * W288 + CPD],
                                 func=Act.Relu, bias=MIN_SIZE, scale=-2.0)
            nc.scalar.activation(out=z96b[:],
                                 in_=peri_sb[0:1, R_H2 * W288:R_H2 * W288 + CPD],
                                 func=Act.Relu, bias=MIN_SIZE, scale=-2.0)
            tt(out=z96a[:], in0=z96a[:], in1=z96b[:], op=Alu.add)
            nc.vector.tensor_reduce(out=acc4[0:1, 2:3], in_=z96a[:],
                                    axis=mybir.AxisListType.X, op=Alu.add)

            # ---- partition reduction via PE, then DMA out ----
            nc.vector.memset(ones[:], 1.0)
            nc.tensor.matmul(out=psum4[:], lhsT=acc4[:], rhs=ones[:],
                             start=True, stop=True)
            nc.scalar.activation(out=red4[0:4, 0:1], in_=psum4[:], func=Act.Copy)
            nc.sync.dma_start(out=out[:], in_=red4[0:4, 0:1])
    nc.compile()
    return nc


def _prep_inputs(pred):
    F = _features(pred)                           # [NR-1, M]
    Fe = np.concatenate([F, F[:, :M // 2]], 1)    # wrap-extended
    in_maps = []
    for d in range(NDEV):
        hank2 = np.empty((NR * NKT, HROW), np.float32)
        for r in range(NR - 1):
            for kt in range(NKT):
                base = d * CPD + 128 * kt
                hank2[r * NKT + kt] = Fe[r, base:base + HROW]
        wrow = np.ones(HROW, np.float32)
        wrow[128] = 0.5          # partition 127 reads Row[1+127]: k=384 dup
        for kt in range(NKT):
            hank2[R_WCOL * NKT + kt] = wrow
        peri2 = np.tile(
            np.vstack([F, np.zeros((1, M), np.float32)])[:, d * CPD:(d + 1) * CPD],
            (1, NKT))
        in_maps.append({'peri': np.ascontiguousarray(peri2), 'hank': hank2})
    return in_maps


def _combine(partials):
    m = float(M)
    S_iou = sum(float(p[0, 0]) for p in partials)
    S_rep = sum(float(p[1, 0]) for p in partials)
    S_size = sum(float(p[2, 0]) for p in partials)
    return np.array((2.0 * S_rep) / (m * (m - 1.0)) + S_size / m
                    + (2.0 * S_iou) / (m * m), dtype=np.float32)


def kernel(pred):
    from concourse import bass_utils
    if 'nc' not in _PROGRAM_CACHE:
        _PROGRAM_CACHE['nc'] = _build_program()
    nc = _PROGRAM_CACHE['nc']
    in_maps = _prep_inputs(pred)
    res = bass_utils.run_bass_kernel_spmd(nc, in_maps, core_ids=list(range(NDEV)))
    return _combine([r['out'] for r in res.results])


if __name__ == '__main__':
    pred = np.load('/root/problem/pred.npy')
    print('kernel total:', kernel(pred))



# revision 3
# speedup vs baseline: 1.9743x; 1.9743x over previous
"""Trainium2 Bass kernel for nn_BoxRepelLoss (rotated-box repel/IoU loss).

Same sort-free Liang-Barsky/Green's-theorem math as the previous version
(2*Area(P inter Q) = sum over the 8 edges of dt * cross(a, e)), but with a
transposed, pruned pair layout:

  partition p = box index i within one of six 128-row blocks,
  free dim    = k-diagonal (partner j = i + k), k = 1..K*.

Host-side preprocessing sorts boxes by center-x. Because dist(i, j) >=
x_j - x_i in sorted order, any diagonal k whose minimum x-gap exceeds
T = max(repel_margin, max_i hypot(w_i, h_i)) contributes exactly zero to
every loss term, so it can be dropped. The host verifies this bound per
input; for the default regime K* = 144 of 384 diagonals survive (2.67x
less pair work). A K = 384 fallback program (always correct, with the
k = 384 diagonal double-count weighted 0.5) is compiled only if the bound
fails.

Work is dealt to the 8 cores as (block, k-chunk) cells: 6 blocks x
(K/WK) chunks of WK diagonals, S = 6K/(8 WK) cells per core, so every
core runs an identical program over [S segments x WK columns] and the
segment->cell meaning lives purely in the host-prepared data. The host
fully materializes each core's SBUF operand image [128, 43W] (hank-side
per-j rows, broadcast per-i rows, and a duplicate-weight row), making the
DMA a plain contiguous copy split into consumption-ordered waves.

The size-penalty term is O(N) and computed on host; each core returns
(S_iou, S_rep) partial sums which the host combines:
  total = 2*S_rep/(m(m-1)) + size_loss + 2*S_iou/m^2
"""

import numpy as np

M = 768
NDEV = 8
NB = 6                    # 128-row i-blocks
REPEL_MARGIN = 0.08
MIN_SIZE = 0.02
IOU_MARGIN = 0.1

# (name, side) slots in column order == DMA consumption order.
# side 'h' = per-j (hankel), 'p' = per-i (broadcast), 'c' = constant row.
_ROWSPEC = [
    ('cos', 'h', 1), ('sin', 'h', 1), ('uc', 'h', 1), ('us', 'h', 1),
    ('xa', 'p', 4), ('ya', 'p', 4),
    ('cos', 'p', 1), ('sin', 'p', 1), ('uc', 'p', 1), ('us', 'p', 1),
    ('xa', 'h', 4), ('ya', 'h', 4),
    ('w2', 'h', 1), ('h2', 'h', 1), ('w2', 'p', 1), ('h2', 'p', 1),
    ('K', 'h', 4), ('K', 'p', 4),
    ('cx', 'h', 1), ('cy', 'h', 1), ('a2', 'h', 1),
    ('cx', 'p', 1), ('cy', 'p', 1), ('a2', 'p', 1),
    ('wt', 'c', 1),
]
_OFF = {}
_c = 0
for _n, _s, _k in _ROWSPEC:
    _OFF[(_n, _s)] = _c
    _c += _k
NROWS = _c                                    # 43 W-unit rows
# wave boundaries (in W units): A-dir1 | A-dir2 | B | C | epilogue
_WAVES = [0, 12, 24, 28, 36, NROWS]

_PROGRAM_CACHE = {}


def _build_program(K, WK):
    import concourse.bass as bass
    import concourse.mybir as mybir
    from concourse import bacc
    from concourse.tile import TileContext

    S = (NB * K) // (8 * WK)
    assert NB * K == 8 * WK * S
    W = S * WK
    W4 = 4 * W
    NCOLS = NROWS * W

    fp32 = mybir.dt.float32
    Alu = mybir.AluOpType
    Act = mybir.ActivationFunctionType

    nc = bacc.Bacc('TRN2', target_bir_lowering=False, debug=False)
    t = nc.alloc_sbuf_tensor(f'const-f32-{REPEL_MARGIN}', [128, 1], fp32)
    nc.gpsimd.memset(t.ap(), REPEL_MARGIN)
    nc.const_aps.aps[(fp32, REPEL_MARGIN)] = t.ap()
    nc.all_engine_barrier()

    img = nc.dram_tensor('img', [128, NCOLS], fp32, kind='ExternalInput')
    out = nc.dram_tensor('out', [2, 1], fp32, kind='ExternalOutput')

    def ap(tile, off, free_dims):
        b = tile[:]
        return bass.AP(b.tensor, b.offset + off, [list(b.ap[0])] + free_dims)

    with TileContext(nc) as tc:
        with tc.tile_pool(name='p', bufs=1) as pool, \
             tc.tile_pool(name='ps', bufs=1, space='PSUM') as ppool:
            psum2 = ppool.tile([2, 1], fp32, tag='psum2')
            sb = pool.tile([128, NCOLS], fp32, tag='img')

            for wi in range(len(_WAVES) - 1):
                c0, c1 = _WAVES[wi] * W, _WAVES[wi + 1] * W
                sbv = sb[:]
                nc.sync.dma_start(
                    out=bass.AP(sbv.tensor, sbv.offset + c0,
                                [list(sbv.ap[0]), [1, c1 - c0]]),
                    in_=bass.AP(img[:].tensor, c0, [[NCOLS, 128], [1, c1 - c0]]))

            def row(name, side, nW=1):          # flat [128, nW*W]
                return ap(sb, _OFF[(name, side)] * W, [[1, nW * W]])

            def ebc(name, side):                # one row e-broadcast [128,4,W]
                return ap(sb, _OFF[(name, side)] * W, [[0, 4], [1, W]])

            def e4(tile, off):                  # [128,4,W] over 4W flat cols
                return ap(tile, off, [[W, 4], [1, W]])

            def fl(tile, off, n):               # flat [128, n]
                return ap(tile, off, [[1, n]])

            DALL = pool.tile([128, 4 * W4], fp32, tag='DALL')
            TMP = pool.tile([128, 4 * W4], fp32, tag='TMP')
            RALL = pool.tile([128, 4 * W4], fp32, tag='RALL')
            RINV = pool.tile([128, 4 * W4], fp32, tag='RINV')
            SS = pool.tile([128, W4], fp32, tag='SS')
            U = pool.tile([128, W], fp32, tag='U')
            DX = pool.tile([128, W], fp32, tag='DX')
            DY = pool.tile([128, W], fp32, tag='DY')
            acc = pool.tile([128, 2], fp32, tag='acc')
            ones = pool.tile([128, 1], fp32, tag='ones')
            red = pool.tile([128, 1], fp32, tag='red')

            tt = nc.vector.tensor_tensor
            ts = nc.vector.tensor_scalar
            stt = nc.vector.scalar_tensor_tensor
            gtt = nc.gpsimd.tensor_tensor

            # ---- A: corner projections dca + edge projections r ----
            # groups in DALL/RALL: [dc1 | ds1 | dc2 | ds2], each 4W wide.
            # dir1: subject i corners vs clip j frame; dir2 swapped.
            for g, (cosr, sinr, xar, yar, ucr, usr) in enumerate((
                    (('cos', 'h'), ('sin', 'h'), ('xa', 'p'), ('ya', 'p'),
                     ('uc', 'h'), ('us', 'h')),
                    (('cos', 'p'), ('sin', 'p'), ('xa', 'h'), ('ya', 'h'),
                     ('uc', 'p'), ('us', 'p')))):
                dc, ds = 2 * g * W4, (2 * g + 1) * W4
                tt(out=e4(DALL, dc), in0=ebc(*cosr), in1=e4(sb, _OFF[xar] * W),
                   op=Alu.mult)
                tt(out=e4(TMP, dc), in0=ebc(*sinr), in1=e4(sb, _OFF[yar] * W),
                   op=Alu.mult)
                tt(out=fl(DALL, dc, W4), in0=fl(DALL, dc, W4),
                   in1=fl(TMP, dc, W4), op=Alu.add)
                tt(out=e4(DALL, dc), in0=e4(DALL, dc), in1=ebc(*ucr),
                   op=Alu.subtract)
                tt(out=e4(DALL, ds), in0=ebc(*cosr), in1=e4(sb, _OFF[yar] * W),
                   op=Alu.mult)
                tt(out=e4(TMP, ds), in0=ebc(*sinr), in1=e4(sb, _OFF[xar] * W),
                   op=Alu.mult)
                tt(out=fl(DALL, ds, W4), in0=fl(DALL, ds, W4),
                   in1=fl(TMP, ds, W4), op=Alu.subtract)
                tt(out=e4(DALL, ds), in0=e4(DALL, ds), in1=ebc(*usr),
                   op=Alu.subtract)

            # r[e] = dca[(e+1)%4] - dca[e], all 4 groups in two ops
            tt(out=ap(RALL, 0, [[W4, 4], [1, 3 * W]]),
               in0=ap(DALL, W, [[W4, 4], [1, 3 * W]]),
               in1=ap(DALL, 0, [[W4, 4], [1, 3 * W]]), op=Alu.subtract)
            tt(out=ap(RALL, 3 * W, [[W4, 4], [1, W]]),
               in0=ap(DALL, 0, [[W4, 4], [1, W]]),
               in1=ap(DALL, 3 * W, [[W4, 4], [1, W]]), op=Alu.subtract)

            # ---- B: slab interval endpoints ----
            nc.vector.reciprocal_approx_fast(out=RINV[:], in_=RALL[:])
            tt(out=DALL[:], in0=DALL[:], in1=RINV[:], op=Alu.mult)  # g
            for g, wr in enumerate((('w2', 'h'), ('h2', 'h'),
                                    ('w2', 'p'), ('h2', 'p'))):
                tt(out=fl(RALL, g * W4, W4), in0=ebc(*wr),
                   in1=fl(RINV, g * W4, W4), op=Alu.mult)           # h
            stt(out=RALL[:], in0=RALL[:], scalar=-1.0, op0=Alu.mult,
                in1=RALL[:], op1=Alu.max)                           # habs
            tt(out=TMP[:], in0=RALL[:], in1=DALL[:], op=Alu.subtract)  # hi
            tt(out=RALL[:], in0=RALL[:], in1=DALL[:], op=Alu.add)      # nlo

            # ---- C: interval intersection, dt, weight, reduce ----
            W8 = 2 * W4
            tt(out=fl(DALL, 0, W8),
               in0=ap(TMP, 0, [[W8, 2], [1, W4]]),
               in1=ap(TMP, W4, [[W8, 2], [1, W4]]), op=Alu.min)     # HI2
            tt(out=fl(DALL, W8, W8),
               in0=ap(RALL, 0, [[W8, 2], [1, W4]]),
               in1=ap(RALL, W4, [[W8, 2], [1, W4]]), op=Alu.min)    # NLO2
            ts(out=fl(DALL, 0, W8), in0=fl(DALL, 0, W8), scalar1=1.0,
               scalar2=None, op0=Alu.min)
            stt(out=fl(TMP, 0, W8), in0=fl(DALL, W8, W8), scalar=0.0,
                op0=Alu.min, in1=fl(DALL, 0, W8), op1=Alu.add)      # dt
            ts(out=fl(TMP, 0, W8), in0=fl(TMP, 0, W8), scalar1=0.0,
               scalar2=None, op0=Alu.max)
            tt(out=fl(TMP, 0, W4), in0=fl(TMP, 0, W4),
               in1=e4(sb, _OFF[('K', 'p')] * W), op=Alu.mult)
            tt(out=fl(TMP, W4, W4), in0=fl(TMP, W4, W4),
               in1=e4(sb, _OFF[('K', 'h')] * W), op=Alu.mult)
            tt(out=SS[:], in0=fl(TMP, 0, W4), in1=fl(TMP, W4, W4), op=Alu.add)
            tt(out=fl(SS, 0, 2 * W), in0=fl(SS, 0, 2 * W),
               in1=fl(SS, 2 * W, 2 * W), op=Alu.add)
            tt(out=fl(SS, 0, W), in0=fl(SS, 0, W), in1=fl(SS, W, W),
               op=Alu.add)                                          # S = 2*inter

            # ---- IoU epilogue ----
            tt(out=U[:], in0=row('a2', 'h'), in1=row('a2', 'p'), op=Alu.add)
            tt(out=U[:], in0=U[:], in1=fl(SS, 0, W), op=Alu.subtract)
            nc.vector.reciprocal_approx_fast(out=U[:], in_=U[:])
            tt(out=U[:], in0=fl(SS, 0, W), in1=U[:], op=Alu.mult)   # iou
            ts(out=U[:], in0=U[:], scalar1=IOU_MARGIN, scalar2=0.0,
               op0=Alu.subtract, op1=Alu.max)
            tt(out=U[:], in0=U[:], in1=row('wt', 'c'), op=Alu.mult)
            nc.vector.tensor_reduce(out=acc[:, 0:1], in_=U[:],
                                    axis=mybir.AxisListType.X, op=Alu.add)

            # ---- repel (GpSimd chain + Scalar sqrt/hinge) ----
            gtt(out=DX[:], in0=row('cx', 'h'), in1=row('cx', 'p'),
                op=Alu.subtract)
            gtt(out=DY[:], in0=row('cy', 'h'), in1=row('cy', 'p'),
                op=Alu.subtract)
            gtt(out=DX[:], in0=DX[:], in1=DX[:], op=Alu.mult)
            gtt(out=DY[:], in0=DY[:], in1=DY[:], op=Alu.mult)
            gtt(out=DX[:], in0=DX[:], in1=DY[:], op=Alu.add)
            nc.scalar.activation(out=DX[:], in_=DX[:], func=Act.Sqrt)
            nc.scalar.activation(out=DX[:], in_=DX[:], func=Act.Relu,
                                 bias=REPEL_MARGIN, scale=-1.0)
            tt(out=DX[:], in0=DX[:], in1=row('wt', 'c'), op=Alu.mult)
            nc.vector.tensor_reduce(out=acc[:, 1:2], in_=DX[:],
                                    axis=mybir.AxisListType.X, op=Alu.add)

            # ---- partition reduction via PE, DMA out ----
            nc.vector.memset(ones[:], 1.0)
            nc.tensor.matmul(out=psum2[:], lhsT=acc[:], rhs=ones[:],
                             start=True, stop=True)
            nc.scalar.activation(out=red[0:2, 0:1], in_=psum2[:], func=Act.Copy)
            nc.sync.dma_start(out=out[:], in_=red[0:2, 0:1])
    nc.compile()
    return nc


def _features(p):
    cx, cy, w, h = p[:, 0], p[:, 1], p[:, 2], p[:, 3]
    th = np.arctan2(p[:, 5], p[:, 4]).astype(np.float32)
    c = np.cos(th).astype(np.float32)
    s = np.sin(th).astype(np.float32)
    dx = np.stack([-w, w, w, -w], 0) * np.float32(0.5)
    dy = np.stack([-h, -h, h, h], 0) * np.float32(0.5)
    xa = cx[None] + c[None] * dx - s[None] * dy
    ya = cy[None] + s[None] * dx + c[None] * dy
    ex = np.roll(xa, -1, 0) - xa
    ey = np.roll(ya, -1, 0) - ya
    return {
        'cos': c, 'sin': s,
        'uc': c * cx + s * cy, 'us': -s * cx + c * cy,
        'w2': w * np.float32(0.5), 'h2': h * np.float32(0.5),
        'cx': cx, 'cy': cy, 'a2': np.float32(2.0) * w * h,
        'xa': xa, 'ya': ya, 'K': xa * ey - ya * ex,
    }


def _choose_params(ps):
    """Smallest supported K covering every non-provably-zero diagonal."""
    xs = ps[:, 0]
    T = float(max(REPEL_MARGIN, np.hypot(ps[:, 2], ps[:, 3]).max()))
    need = 1
    for k in range(1, M // 2 + 1):
        if float((xs[k:] - xs[:-k]).min()) <= T:
            need = k
    if need <= 144:
        return (144, 36)
    return (384, 48)


def _prep_inputs(pred):
    p = np.asarray(pred, np.float32)[:M]
    order = np.argsort(p[:, 0], kind='stable')
    ps = p[order]

    size_pen = (np.maximum(MIN_SIZE - ps[:, 2], 0.0)
                + np.maximum(MIN_SIZE - ps[:, 3], 0.0))
    size_loss = float(size_pen.mean())

    K, WK = _choose_params(ps)
    S = (NB * K) // (8 * WK)
    W = S * WK
    nkc = K // WK

    F = _features(ps)
    Fe = {}
    for n, v in F.items():
        Fe[n] = np.concatenate([v, v[..., :K]], axis=-1)

    pidx = np.arange(128)
    kk = np.arange(1, WK + 1)
    in_maps = []
    for d in range(NDEV):
        cells = np.arange(d * S, (d + 1) * S)
        b = cells // nkc
        kofs = (cells % nkc) * WK
        # [128, S, WK]
        jm = (128 * b[None, :, None] + kofs[None, :, None]
              + pidx[:, None, None] + kk[None, None, :])
        im = (128 * b[None, :, None] + pidx[:, None, None]
              + np.zeros((1, 1, WK), np.int64))
        kval = (kofs[None, :, None] + kk[None, None, :]
                + np.zeros((128, 1, 1), np.int64))
        img = np.empty((128, NROWS * W), np.float32)
        for (n, side, nsl) in _ROWSPEC:
            o = _OFF[(n, side)] * W
            if side == 'c':
                img[:, o:o + W] = np.where(kval == 384, 0.5, 1.0
                                           ).reshape(128, W)
                continue
            idx = jm if side == 'h' else im
            src = Fe[n]
            if nsl == 1:
                img[:, o:o + W] = src[idx].reshape(128, W)
            else:
                for e in range(4):
                    img[:, o + e * W:o + (e + 1) * W] = \
                        src[e][idx].reshape(128, W)
        in_maps.append({'img': img})

    _PROGRAM_CACHE['size_loss'] = size_loss
    key = (K, WK)
    if key not in _PROGRAM_CACHE:
        _PROGRAM_CACHE[key] = _build_program(K, WK)
    _PROGRAM_CACHE['nc'] = _PROGRAM_CACHE[key]
    return in_maps


def _combine(partials):
    m = float(M)
    S_iou = sum(float(p[0, 0]) for p in partials)
    S_rep = sum(float(p[1, 0]) for p in partials)
    return np.array((2.0 * S_rep) / (m * (m - 1.0))
                    + _PROGRAM_CACHE['size_loss']
                    + (2.0 * S_iou) / (m * m), dtype=np.float32)


def kernel(pred):
    from concourse import bass_utils
    in_maps = _prep_inputs(pred)
    nc = _PROGRAM_CACHE['nc']
    res = bass_utils.run_bass_kernel_spmd(nc, in_maps, core_ids=list(range(NDEV)))
    return _combine([r['out'] for r in res.results])


if __name__ == '__main__':
    pred = np.load('/root/problem/pred.npy')
    print('kernel total:', kernel(pred))


# revision 8
# speedup vs baseline: 2.5312x; 1.2820x over previous
"""Trainium2 Bass kernel for nn_BoxRepelLoss (rotated-box repel/IoU loss).

Sort-free Liang-Barsky/Green's-theorem rotated-IoU (2*Area(P inter Q) =
sum over the 8 edges of dt * cross(a, e)) over a transposed, pruned pair
layout:

  partition p = box index i within one of six 128-row blocks,
  free dim    = k-diagonal (partner j = i + k), k = 1..K*.

Host preprocessing sorts boxes by center-x; dist(i,j) >= x_j - x_i then
lets every diagonal k whose minimum x-gap exceeds
T = max(repel_margin, max_i hypot(w_i, h_i)) be dropped as exactly zero.
The bound is verified per input (K* = 144 of 384 survives here; a K=384
fallback program is compiled only if the bound fails, with the k = 384
double-counted diagonal weighted 0.5).

Work goes to the 8 cores as (block, k-chunk) cells, S cells per core, so
all cores run one program over [S segments x WK columns]; segment->cell
meaning lives in host-prepared data. The host fully materializes each
core's SBUF image [128, 39W] in float16, with all positions re-centered
per pair at box i's center (values become O(0.2), making fp16 rounding
~1e-4 absolute; verified 1e-5 total relative error). Centering also
zeroes the dir-2 center-projection terms and yields the repel dx/dy rows
directly. The reciprocal is clamped to +-60000 so fp16 overflow paths
stay NaN-free (far pairs still produce exactly-zero contributions).

fp16 doubles DVE tensor_tensor throughput (2x_1P mode) and quadruples
tensor_scalar, and halves DMA bytes. Final hinge sums are reduced in
fp32. The O(N) size-penalty term is computed on host; cores return
(S_iou, S_rep) partials:
  total = 2*S_rep/(m(m-1)) + size_loss + 2*S_iou/m^2
"""

import numpy as np

M = 768
NDEV = 8
NB = 6                    # 128-row i-blocks
REPEL_MARGIN = 0.08
MIN_SIZE = 0.02
IOU_MARGIN = 0.1
RCLAMP = 60000.0          # fp16-safe reciprocal clamp

# (name, slots) in column order == DMA consumption order.
_ROWSPEC = [
    ('cos_h', 1), ('sin_h', 1), ('uc_h', 1), ('us_h', 1),
    ('xa_p', 4), ('ya_p', 4),
    ('cos_p', 1), ('sin_p', 1), ('xa_h', 4), ('ya_h', 4),
    ('w2_h', 1), ('h2_h', 1), ('w2_p', 1), ('h2_p', 1),
    ('K_p', 4), ('K_h', 4),
    ('dx', 1), ('dy', 1), ('a2_h', 1), ('a2_p', 1), ('wt', 1),
]
_OFF = {}
_c = 0
for _n, _k in _ROWSPEC:
    _OFF[_n] = _c
    _c += _k
NROWS = _c                                    # 39 W-unit rows
# waves: A-dir1 | A-dir2 | B | C | epilogue
_WAVES = [0, 12, 22, 26, 34, NROWS]

_PROGRAM_CACHE = {}


def _build_program(K, WK):
    import concourse.bass as bass
    import concourse.mybir as mybir
    from concourse import bacc
    from concourse.tile import TileContext

    S = (NB * K) // (8 * WK)
    assert NB * K == 8 * WK * S
    W = S * WK
    W4 = 4 * W
    W8 = 2 * W4
    NCOLS = NROWS * W

    fp32 = mybir.dt.float32
    fp16 = mybir.dt.float16
    Alu = mybir.AluOpType
    Act = mybir.ActivationFunctionType

    nc = bacc.Bacc('TRN2', target_bir_lowering=False, debug=False)
    img = nc.dram_tensor('img', [128, NCOLS], fp16, kind='ExternalInput')
    out = nc.dram_tensor('out', [2, 1], fp32, kind='ExternalOutput')

    def ap(tile, off, free_dims):
        b = tile[:]
        return bass.AP(b.tensor, b.offset + off, [list(b.ap[0])] + free_dims)

    with TileContext(nc) as tc:
        with tc.tile_pool(name='p', bufs=1) as pool, \
             tc.tile_pool(name='ps', bufs=1, space='PSUM') as ppool:
            psum2 = ppool.tile([2, 1], fp32, tag='psum2')
            sb = pool.tile([128, NCOLS], fp16, tag='img')

            for wi in range(len(_WAVES) - 1):
                c0, c1 = _WAVES[wi] * W, _WAVES[wi + 1] * W
                sbv = sb[:]
                nc.sync.dma_start(
                    out=bass.AP(sbv.tensor, sbv.offset + c0,
                                [list(sbv.ap[0]), [1, c1 - c0]]),
                    in_=bass.AP(img[:].tensor, c0, [[NCOLS, 128], [1, c1 - c0]]))

            def row(name, nW=1):                # flat [128, nW*W]
                return ap(sb, _OFF[name] * W, [[1, nW * W]])

            def ebc(name):                      # one row e-broadcast [128,4,W]
                return ap(sb, _OFF[name] * W, [[0, 4], [1, W]])

            def e4(tile, off):                  # [128,4,W] over 4W flat cols
                return ap(tile, off, [[W, 4], [1, W]])

            def fl(tile, off, n):               # flat [128, n]
                return ap(tile, off, [[1, n]])

            DALL = pool.tile([128, 4 * W4], fp16, tag='DALL')
            TMP = pool.tile([128, 4 * W4], fp16, tag='TMP')
            RALL = pool.tile([128, 4 * W4], fp16, tag='RALL')
            RF32 = pool.tile([128, 4 * W4], fp32, tag='RF32')
            RINV = pool.tile([128, 4 * W4], fp16, tag='RINV')
            SS = pool.tile([128, W4], fp16, tag='SS')
            S32 = pool.tile([128, W], fp32, tag='S32')
            U = pool.tile([128, W], fp32, tag='U')
            HG = pool.tile([128, W], fp32, tag='HG')
            DX = pool.tile([128, W], fp16, tag='DX')
            DY = pool.tile([128, W], fp16, tag='DY')
            acc = pool.tile([128, 2], fp32, tag='acc')
            ones = pool.tile([128, 1], fp32, tag='ones')
            red = pool.tile([128, 1], fp32, tag='red')

            tt = nc.vector.tensor_tensor
            ts = nc.vector.tensor_scalar
            stt = nc.vector.scalar_tensor_tensor

            # ---- A: corner projections dca + edge projections r ----
            # DALL groups: [dc1 | ds1 | dc2 | ds2]; dir1 = subject-i corners
            # in clip-j frame (centered at i, so dir2 has no center term).
            dc1, ds1, dc2, ds2 = 0, W4, W8, 3 * W4
            tt(out=e4(DALL, dc1), in0=ebc('cos_h'), in1=e4(sb, _OFF['xa_p'] * W),
               op=Alu.mult)
            tt(out=e4(TMP, dc1), in0=ebc('sin_h'), in1=e4(sb, _OFF['ya_p'] * W),
               op=Alu.mult)
            tt(out=fl(DALL, dc1, W4), in0=fl(DALL, dc1, W4),
               in1=fl(TMP, dc1, W4), op=Alu.add)
            tt(out=e4(DALL, dc1), in0=e4(DALL, dc1), in1=ebc('uc_h'),
               op=Alu.subtract)
            tt(out=e4(DALL, ds1), in0=ebc('cos_h'), in1=e4(sb, _OFF['ya_p'] * W),
               op=Alu.mult)
            tt(out=e4(TMP, ds1), in0=ebc('sin_h'), in1=e4(sb, _OFF['xa_p'] * W),
               op=Alu.mult)
            tt(out=fl(DALL, ds1, W4), in0=fl(DALL, ds1, W4),
               in1=fl(TMP, ds1, W4), op=Alu.subtract)
            tt(out=e4(DALL, ds1), in0=e4(DALL, ds1), in1=ebc('us_h'),
               op=Alu.subtract)
            tt(out=e4(DALL, dc2), in0=ebc('cos_p'), in1=e4(sb, _OFF['xa_h'] * W),
               op=Alu.mult)
            tt(out=e4(TMP, dc2), in0=ebc('sin_p'), in1=e4(sb, _OFF['ya_h'] * W),
               op=Alu.mult)
            tt(out=fl(DALL, dc2, W4), in0=fl(DALL, dc2, W4),
               in1=fl(TMP, dc2, W4), op=Alu.add)
            tt(out=e4(DALL, ds2), in0=ebc('cos_p'), in1=e4(sb, _OFF['ya_h'] * W),
               op=Alu.mult)
            tt(out=e4(TMP, ds2), in0=ebc('sin_p'), in1=e4(sb, _OFF['xa_h'] * W),
               op=Alu.mult)
            tt(out=fl(DALL, ds2, W4), in0=fl(DALL, ds2, W4),
               in1=fl(TMP, ds2, W4), op=Alu.subtract)

            # r[e] = dca[(e+1)%4] - dca[e], all 4 groups in two ops
            # (fp32 out: reciprocal_approx needs the fp32 bit layout)
            tt(out=ap(RF32, 0, [[W4, 4], [1, 3 * W]]),
               in0=ap(DALL, W, [[W4, 4], [1, 3 * W]]),
               in1=ap(DALL, 0, [[W4, 4], [1, 3 * W]]), op=Alu.subtract)
            tt(out=ap(RF32, 3 * W, [[W4, 4], [1, W]]),
               in0=ap(DALL, 0, [[W4, 4], [1, W]]),
               in1=ap(DALL, 3 * W, [[W4, 4], [1, W]]), op=Alu.subtract)

            # ---- B: slab interval endpoints ----
            nc.vector.reciprocal_approx_fast(out=RF32[:], in_=RF32[:])
            ts(out=RINV[:], in0=RF32[:], scalar1=RCLAMP, scalar2=-RCLAMP,
               op0=Alu.min, op1=Alu.max)
            tt(out=DALL[:], in0=DALL[:], in1=RINV[:], op=Alu.mult)  # g
            for g, wr in enumerate(('w2_h', 'h2_h', 'w2_p', 'h2_p')):
                tt(out=fl(RALL, g * W4, W4), in0=ebc(wr),
                   in1=fl(RINV, g * W4, W4), op=Alu.mult)           # h
            stt(out=RALL[:], in0=RALL[:], scalar=-1.0, op0=Alu.mult,
                in1=RALL[:], op1=Alu.max)                           # habs
            tt(out=TMP[:], in0=RALL[:], in1=DALL[:], op=Alu.subtract)  # hi
            tt(out=RALL[:], in0=RALL[:], in1=DALL[:], op=Alu.add)      # nlo

            # ---- C: interval intersection, dt, weight, reduce ----
            tt(out=fl(DALL, 0, W8),
               in0=ap(TMP, 0, [[W8, 2], [1, W4]]),
               in1=ap(TMP, W4, [[W8, 2], [1, W4]]), op=Alu.min)     # HI2
            tt(out=fl(DALL, W8, W8),
               in0=ap(RALL, 0, [[W8, 2], [1, W4]]),
               in1=ap(RALL, W4, [[W8, 2], [1, W4]]), op=Alu.min)    # NLO2
            ts(out=fl(DALL, 0, W8), in0=fl(DALL, 0, W8), scalar1=1.0,
               scalar2=None, op0=Alu.min)
            stt(out=fl(TMP, 0, W8), in0=fl(DALL, W8, W8), scalar=0.0,
                op0=Alu.min, in1=fl(DALL, 0, W8), op1=Alu.add)      # dt
            ts(out=fl(TMP, 0, W8), in0=fl(TMP, 0, W8), scalar1=0.0,
               scalar2=None, op0=Alu.max)
            tt(out=fl(TMP, 0, W4), in0=fl(TMP, 0, W4),
               in1=e4(sb, _OFF['K_p'] * W), op=Alu.mult)
            tt(out=fl(TMP, W4, W4), in0=fl(TMP, W4, W4),
               in1=e4(sb, _OFF['K_h'] * W), op=Alu.mult)
            tt(out=SS[:], in0=fl(TMP, 0, W4), in1=fl(TMP, W4, W4), op=Alu.add)
            tt(out=fl(SS, 0, 2 * W), in0=fl(SS, 0, 2 * W),
               in1=fl(SS, 2 * W, 2 * W), op=Alu.add)
            tt(out=fl(SS, 0, W), in0=fl(SS, 0, W), in1=fl(SS, W, W),
               op=Alu.add)                                          # S = 2*inter

            # ---- IoU epilogue (fp32: needs reciprocal) ----
            nc.vector.tensor_copy(out=S32[:], in_=fl(SS, 0, W))
            tt(out=U[:], in0=row('a2_h'), in1=row('a2_p'), op=Alu.add)
            tt(out=U[:], in0=U[:], in1=S32[:], op=Alu.subtract)
            nc.vector.reciprocal_approx_fast(out=U[:], in_=U[:])
            tt(out=U[:], in0=S32[:], in1=U[:], op=Alu.mult)         # iou
            ts(out=HG[:], in0=U[:], scalar1=IOU_MARGIN, scalar2=0.0,
               op0=Alu.subtract, op1=Alu.max)
            if K == 384:
                tt(out=HG[:], in0=HG[:], in1=row('wt'), op=Alu.mult)
            nc.vector.tensor_reduce(out=acc[:, 0:1], in_=HG[:],
                                    axis=mybir.AxisListType.X, op=Alu.add)

            # ---- repel ----
            tt(out=DX[:], in0=row('dx'), in1=row('dx'), op=Alu.mult)
            tt(out=DY[:], in0=row('dy'), in1=row('dy'), op=Alu.mult)
            tt(out=DX[:], in0=DX[:], in1=DY[:], op=Alu.add)
            nc.scalar.activation(out=DX[:], in_=DX[:], func=Act.Sqrt)
            ts(out=HG[:], in0=DX[:], scalar1=-1.0, scalar2=REPEL_MARGIN,
               op0=Alu.mult, op1=Alu.add)
            ts(out=HG[:], in0=HG[:], scalar1=0.0, scalar2=None, op0=Alu.max)
            if K == 384:
                tt(out=HG[:], in0=HG[:], in1=row('wt'), op=Alu.mult)
            nc.vector.tensor_reduce(out=acc[:, 1:2], in_=HG[:],
                                    axis=mybir.AxisListType.X, op=Alu.add)

            # ---- partition reduction via PE, DMA out ----
            nc.vector.memset(ones[:], 1.0)
            nc.tensor.matmul(out=psum2[:], lhsT=acc[:], rhs=ones[:],
                             start=True, stop=True)
            nc.scalar.activation(out=red[0:2, 0:1], in_=psum2[:], func=Act.Copy)
            nc.sync.dma_start(out=out[:], in_=red[0:2, 0:1])
    nc.compile()
    return nc


def _features(p):
    cx, cy, w, h = p[:, 0], p[:, 1], p[:, 2], p[:, 3]
    th = np.arctan2(p[:, 5], p[:, 4]).astype(np.float32)
    c = np.cos(th).astype(np.float32)
    s = np.sin(th).astype(np.float32)
    dx = np.stack([-w, w, w, -w], 0) * np.float32(0.5)
    dy = np.stack([-h, -h, h, h], 0) * np.float32(0.5)
    xa = cx[None] + c[None] * dx - s[None] * dy
    ya = cy[None] + s[None] * dx + c[None] * dy
    return {
        'cos': c, 'sin': s,
        'w2': w * np.float32(0.5), 'h2': h * np.float32(0.5),
        'cx': cx, 'cy': cy, 'a2': np.float32(2.0) * w * h,
        'xa': xa, 'ya': ya,
    }


def _choose_params(ps):
    """Smallest supported K covering every non-provably-zero diagonal."""
    xs = ps[:, 0]
    T = float(max(REPEL_MARGIN, np.hypot(ps[:, 2], ps[:, 3]).max()))
    need = 1
    for k in range(1, M // 2 + 1):
        if float((xs[k:] - xs[:-k]).min()) <= T:
            need = k
    if need <= 144:
        return (144, 36)
    return (384, 48)


def _prep_inputs(pred):
    p = np.asarray(pred, np.float32)[:M]
    order = np.argsort(p[:, 0], kind='stable')
    ps = p[order]

    size_pen = (np.maximum(MIN_SIZE - ps[:, 2], 0.0)
                + np.maximum(MIN_SIZE - ps[:, 3], 0.0))
    size_loss = float(size_pen.mean())

    K, WK = _choose_params(ps)
    S = (NB * K) // (8 * WK)
    W = S * WK
    nkc = K // WK

    F = _features(ps)
    Fe = {n: np.concatenate([v, v[..., :K]], axis=-1) for n, v in F.items()}

    pidx = np.arange(128)
    kk = np.arange(1, WK + 1)
    in_maps = []
    for d in range(NDEV):
        cells = np.arange(d * S, (d + 1) * S)
        b = cells // nkc
        kofs = (cells % nkc) * WK
        jm = (128 * b[None, :, None] + kofs[None, :, None]
              + pidx[:, None, None] + kk[None, None, :]).reshape(128, W)
        im = (128 * b[None, :, None] + pidx[:, None, None]
              + np.zeros((1, 1, WK), np.int64)).reshape(128, W)
        kval = (kofs[None, :, None] + kk[None, None, :]
                + np.zeros((128, 1, 1), np.int64)).reshape(128, W)

        cxi, cyi = Fe['cx'][im], Fe['cy'][im]
        rows = {
            'cos_h': Fe['cos'][jm], 'sin_h': Fe['sin'][jm],
            'cos_p': Fe['cos'][im], 'sin_p': Fe['sin'][im],
            'w2_h': Fe['w2'][jm], 'h2_h': Fe['h2'][jm],
            'w2_p': Fe['w2'][im], 'h2_p': Fe['h2'][im],
            'a2_h': Fe['a2'][jm], 'a2_p': Fe['a2'][im],
            'dx': Fe['cx'][jm] - cxi, 'dy': Fe['cy'][jm] - cyi,
            'wt': np.where(kval == 384, np.float32(0.5), np.float32(1.0)),
        }
        rows['uc_h'] = rows['cos_h'] * rows['dx'] + rows['sin_h'] * rows['dy']
        rows['us_h'] = -rows['sin_h'] * rows['dx'] + rows['cos_h'] * rows['dy']
        xap = [Fe['xa'][e][im] - cxi for e in range(4)]
        yap = [Fe['ya'][e][im] - cyi for e in range(4)]
        xah = [Fe['xa'][e][jm] - cxi for e in range(4)]
        yah = [Fe['ya'][e][jm] - cyi for e in range(4)]
        rows['xa_p'], rows['ya_p'] = xap, yap
        rows['xa_h'], rows['ya_h'] = xah, yah
        rows['K_p'] = [xap[e] * yap[(e + 1) % 4] - yap[e] * xap[(e + 1) % 4]
                       for e in range(4)]
        rows['K_h'] = [xah[e] * yah[(e + 1) % 4] - yah[e] * xah[(e + 1) % 4]
                       for e in range(4)]

        img = np.empty((128, NROWS * W), np.float16)
        for (n, nsl) in _ROWSPEC:
            o = _OFF[n] * W
            if nsl == 1:
                img[:, o:o + W] = rows[n].astype(np.float16)
            else:
                for e in range(4):
                    img[:, o + e * W:o + (e + 1) * W] = \
                        rows[n][e].astype(np.float16)
        in_maps.append({'img': img})

    _PROGRAM_CACHE['size_loss'] = size_loss
    key = (K, WK)
    if key not in _PROGRAM_CACHE:
        _PROGRAM_CACHE[key] = _build_program(K, WK)
    _PROGRAM_CACHE['nc'] = _PROGRAM_CACHE[key]
    return in_maps


def _combine(partials):
    m = float(M)
    S_iou = sum(float(p[0, 0]) for p in partials)
    S_rep = sum(float(p[1, 0]) for p in partials)
    return np.array((2.0 * S_rep) / (m * (m - 1.0))
                    + _PROGRAM_CACHE['size_loss']
                    + (2.0 * S_iou) / (m * m), dtype=np.float32)


def kernel(pred):
    from concourse import bass_utils
    in_maps = _prep_inputs(pred)
    nc = _PROGRAM_CACHE['nc']
    res = bass_utils.run_bass_kernel_spmd(nc, in_maps, core_ids=list(range(NDEV)))
    return _combine([r['out'] for r in res.results])


if __name__ == '__main__':
    pred = np.load('/root/problem/pred.npy')
    print('kernel total:', kernel(pred))


# revision 12
# speedup vs baseline: 2.5731x; 1.0165x over previous
"""Trainium2 Bass kernel for nn_BoxRepelLoss (rotated-box repel/IoU loss).

Sort-free Liang-Barsky/Green's-theorem rotated-IoU (2*Area(P inter Q) =
sum over the 8 edges of dt * cross(a, e)) over a transposed, pruned pair
layout:

  partition p = box index i within one of six 128-row blocks,
  free dim    = k-diagonal (partner j = i + k), k = 1..K*.

Host preprocessing sorts boxes by center-x; dist(i,j) >= x_j - x_i then
lets every diagonal k whose minimum x-gap exceeds
T = max(repel_margin, max_i hypot(w_i, h_i)) be dropped as exactly zero.
The bound is verified per input (K* = 144 of 384 survives here; a K=384
fallback program is compiled only if the bound fails, with the k = 384
double-counted diagonal weighted 0.5).

Work goes to the 8 cores as (block, k-chunk) cells, S cells per core, so
all cores run one program over [S segments x WK columns]; segment->cell
meaning lives in host-prepared data. The host fully materializes each
core's SBUF image [128, 39W] in float16, with all positions re-centered
per pair at box i's center (values become O(0.2), making fp16 rounding
~1e-4 absolute). Centering also zeroes the dir-2 center-projection terms
and yields the repel dx/dy rows directly.

Engine split: DVE does the wide fp16 tensor work (2x_1P mode);
the Scalar engine does Reciprocal/Abs/Sqrt activations off the critical
path; reciprocals are clamped to +-60000 on the DVE so fp16 overflow
paths stay NaN-free (far pairs still produce exactly-zero dt). DMA runs
as three cascaded waves (each overlaps the previous wave's last column
to force sequential completion), so the first compute wave is not
delayed by packet round-robin with later waves.

Final hinge sums are reduced in fp32. The O(N) size-penalty term is
computed on host; cores return (S_iou, S_rep) partials:
  total = 2*S_rep/(m(m-1)) + size_loss + 2*S_iou/m^2
"""

import numpy as np

M = 768
NDEV = 8
NB = 6                    # 128-row i-blocks
REPEL_MARGIN = 0.08
MIN_SIZE = 0.02
IOU_MARGIN = 0.1
RCLAMP = 60000.0          # fp16-safe reciprocal clamp

# (name, slots) in column order == DMA consumption order.
_ROWSPEC = [
    ('cos_h', 1), ('xa_p', 4), ('sin_h', 1), ('ya_p', 4),
    ('uc_h', 1), ('us_h', 1), ('cos_p', 1), ('sin_p', 1),
    ('xa_h', 4), ('ya_h', 4),
    ('w2_h', 1), ('h2_h', 1), ('w2_p', 1), ('h2_p', 1),
    ('K_p', 4), ('K_h', 4),
    ('dx', 1), ('dy', 1), ('a2_h', 1), ('a2_p', 1), ('wt', 1),
]
_OFF = {}
_c = 0
for _n, _k in _ROWSPEC:
    _OFF[_n] = _c
    _c += _k
NROWS = _c                                    # 39 W-unit rows
_WAVES = [0, 10, 22, NROWS]                   # cascaded priority classes

_PROGRAM_CACHE = {}


def _build_program(K, WK):
    import concourse.bass as bass
    import concourse.mybir as mybir
    from concourse import bacc
    from concourse.tile import TileContext

    S = (NB * K) // (8 * WK)
    assert NB * K == 8 * WK * S
    W = S * WK
    W4 = 4 * W
    W8 = 2 * W4
    NCOLS = NROWS * W

    fp32 = mybir.dt.float32
    fp16 = mybir.dt.float16
    Alu = mybir.AluOpType
    Act = mybir.ActivationFunctionType

    nc = bacc.Bacc('TRN2', target_bir_lowering=False, debug=False)
    img = nc.dram_tensor('img', [128, NCOLS], fp16, kind='ExternalInput')
    out = nc.dram_tensor('out', [2, 1], fp32, kind='ExternalOutput')

    def ap(tile, off, free_dims):
        b = tile[:]
        return bass.AP(b.tensor, b.offset + off, [list(b.ap[0])] + free_dims)

    with TileContext(nc) as tc:
        with tc.tile_pool(name='p', bufs=1) as pool, \
             tc.tile_pool(name='ps', bufs=1, space='PSUM') as ppool:
            psum2 = ppool.tile([2, 1], fp32, tag='psum2')
            sb = pool.tile([128, NCOLS], fp16, tag='img')

            for wi in range(len(_WAVES) - 1):
                c0, c1 = _WAVES[wi] * W, _WAVES[wi + 1] * W
                if wi > 0:
                    c0 -= 1          # overlap previous wave -> cascade order
                sbv = sb[:]
                nc.sync.dma_start(
                    out=bass.AP(sbv.tensor, sbv.offset + c0,
                                [list(sbv.ap[0]), [1, c1 - c0]]),
                    in_=bass.AP(img[:].tensor, c0, [[NCOLS, 128], [1, c1 - c0]]))

            def row(name, nW=1):                # flat [128, nW*W]
                return ap(sb, _OFF[name] * W, [[1, nW * W]])

            def ebc(name):                      # one row e-broadcast [128,4,W]
                return ap(sb, _OFF[name] * W, [[0, 4], [1, W]])

            def e4(tile, off):                  # [128,4,W] over 4W flat cols
                return ap(tile, off, [[W, 4], [1, W]])

            def fl(tile, off, n):               # flat [128, n]
                return ap(tile, off, [[1, n]])

            DALL = pool.tile([128, 4 * W4], fp16, tag='DALL')
            TMP = pool.tile([128, 4 * W4], fp16, tag='TMP')
            RALL = pool.tile([128, 4 * W4], fp16, tag='RALL')
            RF32 = pool.tile([128, 4 * W4], fp32, tag='RF32')
            RINV = pool.tile([128, 4 * W4], fp16, tag='RINV')
            RABS = pool.tile([128, 4 * W4], fp16, tag='RABS')
            SS = pool.tile([128, W4], fp16, tag='SS')
            S32 = pool.tile([128, W], fp32, tag='S32')
            U = pool.tile([128, W], fp32, tag='U')
            HG = pool.tile([128, W], fp32, tag='HG')
            DX = pool.tile([128, W], fp16, tag='DX')
            DY = pool.tile([128, W], fp16, tag='DY')
            acc = pool.tile([128, 2], fp32, tag='acc')
            ones = pool.tile([128, 1], fp32, tag='ones')
            red = pool.tile([128, 1], fp32, tag='red')

            tt = nc.vector.tensor_tensor
            ts = nc.vector.tensor_scalar
            act = nc.scalar.activation

            # ---- A: corner projections dca + edge projections r ----
            # DALL groups: [dc1 | ds1 | dc2 | ds2]; dir1 = subject-i corners
            # in clip-j frame (centered at i, so dir2 has no center term).
            dc1, ds1, dc2, ds2 = 0, W4, W8, 3 * W4
            tt(out=e4(DALL, dc1), in0=ebc('cos_h'), in1=e4(sb, _OFF['xa_p'] * W),
               op=Alu.mult)
            tt(out=e4(TMP, dc1), in0=ebc('sin_h'), in1=e4(sb, _OFF['ya_p'] * W),
               op=Alu.mult)
            tt(out=fl(DALL, dc1, W4), in0=fl(DALL, dc1, W4),
               in1=fl(TMP, dc1, W4), op=Alu.add)
            tt(out=e4(DALL, dc1), in0=e4(DALL, dc1), in1=ebc('uc_h'),
               op=Alu.subtract)
            tt(out=e4(DALL, ds1), in0=ebc('cos_h'), in1=e4(sb, _OFF['ya_p'] * W),
               op=Alu.mult)
            tt(out=e4(TMP, ds1), in0=ebc('sin_h'), in1=e4(sb, _OFF['xa_p'] * W),
               op=Alu.mult)
            tt(out=fl(DALL, ds1, W4), in0=fl(DALL, ds1, W4),
               in1=fl(TMP, ds1, W4), op=Alu.subtract)
            tt(out=e4(DALL, ds1), in0=e4(DALL, ds1), in1=ebc('us_h'),
               op=Alu.subtract)
            tt(out=e4(DALL, dc2), in0=ebc('cos_p'), in1=e4(sb, _OFF['xa_h'] * W),
               op=Alu.mult)
            tt(out=e4(TMP, dc2), in0=ebc('sin_p'), in1=e4(sb, _OFF['ya_h'] * W),
               op=Alu.mult)
            tt(out=fl(DALL, dc2, W4), in0=fl(DALL, dc2, W4),
               in1=fl(TMP, dc2, W4), op=Alu.add)
            tt(out=e4(DALL, ds2), in0=ebc('cos_p'), in1=e4(sb, _OFF['ya_h'] * W),
               op=Alu.mult)
            tt(out=e4(TMP, ds2), in0=ebc('sin_p'), in1=e4(sb, _OFF['xa_h'] * W),
               op=Alu.mult)
            tt(out=fl(DALL, ds2, W4), in0=fl(DALL, ds2, W4),
               in1=fl(TMP, ds2, W4), op=Alu.subtract)

            # r[e] = dca[(e+1)%4] - dca[e], all 4 groups in two ops
            # (fp32 out: reciprocal_approx needs the fp32 bit layout)
            tt(out=ap(RF32, 0, [[W4, 4], [1, 3 * W]]),
               in0=ap(DALL, W, [[W4, 4], [1, 3 * W]]),
               in1=ap(DALL, 0, [[W4, 4], [1, 3 * W]]), op=Alu.subtract)
            tt(out=ap(RF32, 3 * W, [[W4, 4], [1, W]]),
               in0=ap(DALL, 0, [[W4, 4], [1, W]]),
               in1=ap(DALL, 3 * W, [[W4, 4], [1, W]]), op=Alu.subtract)

            # ---- B: slab interval endpoints ----
            nc.vector.reciprocal_approx_fast(out=RF32[:], in_=RF32[:])
            # repel distance chain fills DVE slack around the handoffs
            tt(out=DX[:], in0=row('dx'), in1=row('dx'), op=Alu.mult)
            tt(out=DY[:], in0=row('dy'), in1=row('dy'), op=Alu.mult)
            ts(out=RINV[:], in0=RF32[:], scalar1=RCLAMP, scalar2=-RCLAMP,
               op0=Alu.min, op1=Alu.max)
            act(out=RABS[:], in_=RINV[:], func=Act.Abs)
            tt(out=DX[:], in0=DX[:], in1=DY[:], op=Alu.add)
            tt(out=DALL[:], in0=DALL[:], in1=RINV[:], op=Alu.mult)  # g
            for g, wr in enumerate(('w2_h', 'h2_h', 'w2_p', 'h2_p')):
                tt(out=fl(RALL, g * W4, W4), in0=ebc(wr),
                   in1=fl(RABS, g * W4, W4), op=Alu.mult)           # habs
            act(out=DX[:], in_=DX[:], func=Act.Sqrt)                # dist
            tt(out=TMP[:], in0=RALL[:], in1=DALL[:], op=Alu.subtract)  # hi
            tt(out=RALL[:], in0=RALL[:], in1=DALL[:], op=Alu.add)      # nlo

            # ---- C: interval intersection, dt, weight, reduce ----
            tt(out=fl(DALL, 0, W8),
               in0=ap(TMP, 0, [[W8, 2], [1, W4]]),
               in1=ap(TMP, W4, [[W8, 2], [1, W4]]), op=Alu.min)     # HI2
            tt(out=fl(DALL, W8, W8),
               in0=ap(RALL, 0, [[W8, 2], [1, W4]]),
               in1=ap(RALL, W4, [[W8, 2], [1, W4]]), op=Alu.min)    # NLO2
            ts(out=fl(DALL, 0, W8), in0=fl(DALL, 0, W8), scalar1=1.0,
               scalar2=None, op0=Alu.min)
            ts(out=fl(DALL, W8, W8), in0=fl(DALL, W8, W8), scalar1=0.0,
               scalar2=None, op0=Alu.min)
            tt(out=fl(TMP, 0, W8), in0=fl(DALL, 0, W8),
               in1=fl(DALL, W8, W8), op=Alu.add)                    # dt
            ts(out=fl(TMP, 0, W8), in0=fl(TMP, 0, W8), scalar1=0.0,
               scalar2=None, op0=Alu.max)
            tt(out=fl(TMP, 0, W4), in0=fl(TMP, 0, W4),
               in1=e4(sb, _OFF['K_p'] * W), op=Alu.mult)
            tt(out=fl(TMP, W4, W4), in0=fl(TMP, W4, W4),
               in1=e4(sb, _OFF['K_h'] * W), op=Alu.mult)
            tt(out=SS[:], in0=fl(TMP, 0, W4), in1=fl(TMP, W4, W4), op=Alu.add)
            tt(out=fl(SS, 0, 2 * W), in0=fl(SS, 0, 2 * W),
               in1=fl(SS, 2 * W, 2 * W), op=Alu.add)
            tt(out=fl(SS, 0, W), in0=fl(SS, 0, W), in1=fl(SS, W, W),
               op=Alu.add)                                          # S = 2*inter

            # ---- epilogues ----
            nc.vector.tensor_copy(out=S32[:], in_=fl(SS, 0, W))
            tt(out=U[:], in0=row('a2_h'), in1=row('a2_p'), op=Alu.add)
            tt(out=U[:], in0=U[:], in1=S32[:], op=Alu.subtract)
            nc.vector.reciprocal_approx_fast(out=U[:], in_=U[:])
            # repel hinge interleaved with the iou chain
            ts(out=HG[:], in0=DX[:], scalar1=-1.0, scalar2=REPEL_MARGIN,
               op0=Alu.mult, op1=Alu.add)
            tt(out=U[:], in0=S32[:], in1=U[:], op=Alu.mult)         # iou
            ts(out=HG[:], in0=HG[:], scalar1=0.0, scalar2=None, op0=Alu.max)
            if K == 384:
                tt(out=HG[:], in0=HG[:], in1=row('wt'), op=Alu.mult)
            nc.vector.tensor_reduce(out=acc[:, 1:2], in_=HG[:],
                                    axis=mybir.AxisListType.X, op=Alu.add)
            ts(out=HG[:], in0=U[:], scalar1=IOU_MARGIN, scalar2=0.0,
               op0=Alu.subtract, op1=Alu.max)
            if K == 384:
                tt(out=HG[:], in0=HG[:], in1=row('wt'), op=Alu.mult)
            nc.vector.tensor_reduce(out=acc[:, 0:1], in_=HG[:],
                                    axis=mybir.AxisListType.X, op=Alu.add)

            # ---- partition reduction via PE, DMA out ----
            nc.vector.memset(ones[:], 1.0)
            nc.tensor.matmul(out=psum2[:], lhsT=acc[:], rhs=ones[:],
                             start=True, stop=True)
            act(out=red[0:2, 0:1], in_=psum2[:], func=Act.Copy)
            nc.sync.dma_start(out=out[:], in_=red[0:2, 0:1])
    nc.compile()
    return nc


def _features(p):
    cx, cy, w, h = p[:, 0], p[:, 1], p[:, 2], p[:, 3]
    th = np.arctan2(p[:, 5], p[:, 4]).astype(np.float32)
    c = np.cos(th).astype(np.float32)
    s = np.sin(th).astype(np.float32)
    dx = np.stack([-w, w, w, -w], 0) * np.float32(0.5)
    dy = np.stack([-h, -h, h, h], 0) * np.float32(0.5)
    xa = cx[None] + c[None] * dx - s[None] * dy
    ya = cy[None] + s[None] * dx + c[None] * dy
    return {
        'cos': c, 'sin': s,
        'w2': w * np.float32(0.5), 'h2': h * np.float32(0.5),
        'cx': cx, 'cy': cy, 'a2': np.float32(2.0) * w * h,
        'xa': xa, 'ya': ya,
    }


def _choose_params(ps):
    """Smallest supported K covering every non-provably-zero diagonal."""
    xs = ps[:, 0]
    T = float(max(REPEL_MARGIN, np.hypot(ps[:, 2], ps[:, 3]).max()))
    need = 1
    for k in range(1, M // 2 + 1):
        if float((xs[k:] - xs[:-k]).min()) <= T:
            need = k
    if need <= 144:
        return (144, 36)
    return (384, 48)


def _prep_inputs(pred):
    p = np.asarray(pred, np.float32)[:M]
    order = np.argsort(p[:, 0], kind='stable')
    ps = p[order]

    size_pen = (np.maximum(MIN_SIZE - ps[:, 2], 0.0)
                + np.maximum(MIN_SIZE - ps[:, 3], 0.0))
    size_loss = float(size_pen.mean())

    K, WK = _choose_params(ps)
    S = (NB * K) // (8 * WK)
    W = S * WK
    nkc = K // WK

    F = _features(ps)
    Fe = {n: np.concatenate([v, v[..., :K]], axis=-1) for n, v in F.items()}

    pidx = np.arange(128)
    kk = np.arange(1, WK + 1)
    in_maps = []
    for d in range(NDEV):
        cells = np.arange(d * S, (d + 1) * S)
        b = cells // nkc
        kofs = (cells % nkc) * WK
        jm = (128 * b[None, :, None] + kofs[None, :, None]
              + pidx[:, None, None] + kk[None, None, :]).reshape(128, W)
        im = (128 * b[None, :, None] + pidx[:, None, None]
              + np.zeros((1, 1, WK), np.int64)).reshape(128, W)
        kval = (kofs[None, :, None] + kk[None, None, :]
                + np.zeros((128, 1, 1), np.int64)).reshape(128, W)

        cxi, cyi = Fe['cx'][im], Fe['cy'][im]
        rows = {
            'cos_h': Fe['cos'][jm], 'sin_h': Fe['sin'][jm],
            'cos_p': Fe['cos'][im], 'sin_p': Fe['sin'][im],
            'w2_h': Fe['w2'][jm], 'h2_h': Fe['h2'][jm],
            'w2_p': Fe['w2'][im], 'h2_p': Fe['h2'][im],
            'a2_h': Fe['a2'][jm], 'a2_p': Fe['a2'][im],
            'dx': Fe['cx'][jm] - cxi, 'dy': Fe['cy'][jm] - cyi,
            'wt': np.where(kval == 384, np.float32(0.5), np.float32(1.0)),
        }
        rows['uc_h'] = rows['cos_h'] * rows['dx'] + rows['sin_h'] * rows['dy']
        rows['us_h'] = -rows['sin_h'] * rows['dx'] + rows['cos_h'] * rows['dy']
        xap = [Fe['xa'][e][im] - cxi for e in range(4)]
        yap = [Fe['ya'][e][im] - cyi for e in range(4)]
        xah = [Fe['xa'][e][jm] - cxi for e in range(4)]
        yah = [Fe['ya'][e][jm] - cyi for e in range(4)]
        rows['xa_p'], rows['ya_p'] = xap, yap
        rows['xa_h'], rows['ya_h'] = xah, yah
        rows['K_p'] = [xap[e] * yap[(e + 1) % 4] - yap[e] * xap[(e + 1) % 4]
                       for e in range(4)]
        rows['K_h'] = [xah[e] * yah[(e + 1) % 4] - yah[e] * xah[(e + 1) % 4]
                       for e in range(4)]

        img = np.empty((128, NROWS * W), np.float16)
        for (n, nsl) in _ROWSPEC:
            o = _OFF[n] * W
            if nsl == 1:
                img[:, o:o + W] = rows[n].astype(np.float16)
            else:
                for e in range(4):
                    img[:, o + e * W:o + (e + 1) * W] = \
                        rows[n][e].astype(np.float16)
        in_maps.append({'img': img})

    _PROGRAM_CACHE['size_loss'] = size_loss
    key = (K, WK)
    if key not in _PROGRAM_CACHE:
        _PROGRAM_CACHE[key] = _build_program(K, WK)
    _PROGRAM_CACHE['nc'] = _PROGRAM_CACHE[key]
    return in_maps


def _combine(partials):
    m = float(M)
    S_iou = sum(float(p[0, 0]) for p in partials)
    S_rep = sum(float(p[1, 0]) for p in partials)
    return np.array((2.0 * S_rep) / (m * (m - 1.0))
                    + _PROGRAM_CACHE['size_loss']
                    + (2.0 * S_iou) / (m * m), dtype=np.float32)


def kernel(pred):
    from concourse import bass_utils
    in_maps = _prep_inputs(pred)
    nc = _PROGRAM_CACHE['nc']
    res = bass_utils.run_bass_kernel_spmd(nc, in_maps, core_ids=list(range(NDEV)))
    return _combine([r['out'] for r in res.results])


if __name__ == '__main__':
    pred = np.load('/root/problem/pred.npy')
    print('kernel total:', kernel(pred))
